# revision 20
# baseline (speedup 1.0000x reference)
"""Trainium2 Bass kernel for a FlowNet-style MPI correlation layer.

Reference computation (per batch b, shift s=(k,i,j), position p=(y,x,z)):
    cost[b,s,p]  = mean_c f1[b,c,p] * f2pad[b,c,p+delta_s]        (243 shifts)
    cmask[b,s,p] = clip(mask1[b,p] * m2pad[b,p+delta_s], 0, 1)
with mask1 = clip(sum_z alpha1, 0, 1) broadcast over z, f2 zero-padded,
m2 one-padded.

Strategy (8 NeuronCores, spatial shard over h: 12 rows/core):
  - Cost path: bf16 elementwise products on VectorE (2x_1P perf mode), with
    c=64 channels on the partition axis packed as (y-half, c) -> 128
    partitions.  Channel reduction runs on TensorE: ones-pattern weights
    reduce each 64-row channel group, with 16 accumulating matmuls packing a
    full 128x512 PSUM bank (32-row strips via tile_position).  ScalarE copies
    PSUM->SBUF; the 1/64 mean scale is folded into the bf16 cast of f1 on the
    host (exact: power of two).
  - dz=+1 shifts would mis-align bf16 pair-packed reads (odd element offset),
    so the host ships a second f2 copy pre-shifted by one z element; shifts
    are processed in dz-phases so one f2 halo buffer is live at a time and
    refills double-buffer.
  - Mask path: bf16 muls with (b, y) on partitions; dy shifts are realized
    with small SBUF->SBUF partition-shift DMAs, dx/dz as free-dim offsets;
    ScalarE upcasts to fp32 before the store.  Mask work is interleaved
    between cost groups in program order so either path fills the other's
    stalls on VectorE.
"""

import numpy as np
import ml_dtypes
from contextlib import ExitStack

import concourse.bacc as bacc
import concourse.tile as tile
from concourse import mybir
from concourse.bass_utils import run_bass_kernel_spmd

# Problem shape (hardcoded per contest contract).
B, C, H, W, D = 4, 64, 96, 96, 8
S, SD = 4, 1                       # spatial / depth search range
NSX = 2 * S + 1                    # 9 shifts per spatial axis
NSD = 2 * SD + 1                   # 3 depth shifts
NS = NSX * NSX * NSD               # 243 total shifts
NSX2 = NSX * NSX
NCORES = 8
HS = H // NCORES                   # 12 rows of y per core
GH = HS // 2                       # 6 = y-half height (partition packing)
HP = HS + 2 * S                    # 20 = y rows incl halo
F2Y, F2X, F2Z = GH + 2 * S, W + 2 * S, D + 2 * SD   # 14, 104, 10
POS = GH * W * D                   # 4608 free positions per product tile
CHUNK = 512
NCHUNK = POS // CHUNK              # 9 matmul chunks per product tile
SGRP = 7                           # shifts per PSUM bank fill (18*7=126 rows)
MSUB = 3                           # mask shifts per staging tile

F32 = mybir.dt.float32
BF16 = mybir.dt.bfloat16


def _mask_steps(nc, tc, ctx, a1, a2py, cmask):
    """Generator: emits mask-path setup, then yields after each 3-shift
    subgroup (81 yields).  Driven interleaved from the cost loop."""
    singles = ctx.enter_context(tc.tile_pool(name="msk_singles", bufs=1))
    m2y_pool = ctx.enter_context(tc.tile_pool(name="msk_m2y", bufs=2))
    mstg_pool = ctx.enter_context(tc.tile_pool(name="msk_stg", bufs=2))

    # mask1 = clip(sum_z alpha1, 0, 1), broadcast over z. partitions b*12+y.
    a1t = singles.tile([B * HS, W, D], F32)
    for b in range(B):
        nc.sync.dma_start(a1t[b * HS:(b + 1) * HS, :, :], a1[b])
    mask1 = singles.tile([B * HS, W], F32)
    nc.vector.tensor_reduce(mask1[:, :], a1t[:, :, :],
                            axis=mybir.AxisListType.X, op=mybir.AluOpType.add)
    nc.vector.tensor_scalar_min(mask1[:, :], mask1[:, :], 1.0)
    mask1b = singles.tile([B * HS, W, D], BF16)
    for zi in range(D):
        nc.vector.tensor_copy(mask1b[:, :, zi], mask1[:, :])

    # m2t: one-padded mask2 halo, partitions b*20+y' (y' in padded grid).
    # a2py is alpha2 with y pre-padded by 0.125 (padded rows z-sum to 1.0);
    # x/z pads come from the memset(1.0).
    a2t = singles.tile([B * HP, W, D], F32)
    for b in range(B):
        nc.sync.dma_start(a2t[b * HP:(b + 1) * HP, :, :], a2py[b])
    m2sum = singles.tile([B * HP, W], F32)
    nc.vector.tensor_reduce(m2sum[:, :], a2t[:, :, :],
                            axis=mybir.AxisListType.X, op=mybir.AluOpType.add)
    nc.vector.tensor_scalar_min(m2sum[:, :], m2sum[:, :], 1.0)
    m2t = singles.tile([B * HP, F2X, F2Z], F32)
    nc.vector.memset(m2t[:, :, :], 1.0)
    for zi in range(D):
        nc.vector.tensor_copy(m2t[:, S:S + W, SD + zi], m2sum[:, :])
    # bf16 copies: plain + z-shifted-by-1 (keeps pair-packed 2x DVE mode
    # for odd dz reads).
    m2tb = singles.tile([B * HP, F2X, F2Z], BF16)
    m2tbz = singles.tile([B * HP, F2X, F2Z], BF16)
    nc.vector.tensor_copy(m2tb[:, :, :], m2t[:, :, :])
    nc.vector.tensor_copy(m2tbz[:, :, :F2Z - 1], m2t[:, :, 1:])
    nc.vector.memset(m2tbz[:, :, F2Z - 1], 1.0)

    tiles = {}

    def load(dyi):
        m2yt = m2y_pool.tile([B * HS, F2X, F2Z], BF16, tag="m2ya")
        m2ytz = m2y_pool.tile([B * HS, F2X, F2Z], BF16, tag="m2yb")
        for b in range(B):
            nc.gpsimd.dma_start(m2yt[b * HS:(b + 1) * HS, :, :],
                                m2tb[b * HP + dyi:b * HP + dyi + HS, :, :])
            nc.gpsimd.dma_start(m2ytz[b * HS:(b + 1) * HS, :, :],
                                m2tbz[b * HP + dyi:b * HP + dyi + HS, :, :])
        tiles[dyi] = (m2yt, m2ytz)

    load(0)
    for dyi in range(NSX):
        if dyi + 1 < NSX:
            load(dyi + 1)
        m2yt, m2ytz = tiles.pop(dyi)
        for dzi in range(NSD):
            msrc, dz0 = (m2yt, dzi) if dzi != 1 else (m2ytz, 0)
            for dx0 in range(0, NSX, MSUB):
                mstg = mstg_pool.tile([B * HS, MSUB, W, D], BF16, tag="mstgb")
                for dd in range(MSUB):
                    nc.vector.tensor_mul(
                        mstg[:, dd, :, :], mask1b[:, :, :],
                        msrc[:, dx0 + dd:dx0 + dd + W, dz0:dz0 + D])
                mstf = mstg_pool.tile([B * HS, MSUB, W, D], F32, tag="mstgf")
                nc.scalar.copy(mstf[:, :, :, :], mstg[:, :, :, :])
                s0 = dzi * NSX2 + dyi * NSX + dx0
                for b in range(B):
                    hbm = cmask[b, s0:s0 + MSUB].rearrange(
                        "s y x z -> y s (x z)")
                    nc.sync.dma_start(
                        hbm, mstf[b * HS:(b + 1) * HS].rearrange(
                            "p s x z -> p s (x z)"))
                yield


def _build_cost_path(nc, tc, ctx, f1, f2a, f2b, wones, cost, mask_iter):
    """Cost volume: bf16 muls on DVE, channel-reduce on PE.  Drives one mask
    subgroup per cost group so the two paths interleave on VectorE."""
    singles = ctx.enter_context(tc.tile_pool(name="cst_singles", bufs=1))
    f2_pool = ctx.enter_context(tc.tile_pool(name="cst_f2", bufs=2))
    prod_pool = ctx.enter_context(tc.tile_pool(name="cst_prod", bufs=3))
    psum_pool = ctx.enter_context(
        tc.tile_pool(name="cst_psum", bufs=4, space="PSUM"))
    stage_pool = ctx.enter_context(tc.tile_pool(name="cst_stage", bufs=3))

    wt = singles.tile([128, CHUNK], BF16)
    nc.sync.dma_start(wt[:, :], wones[:, :])

    # f1 resident for all b: partition g*64+c (g = y half), free (b, yl, x, z)
    f1t = singles.tile([128, B, GH, W, D], BF16)
    for g in range(2):
        nc.sync.dma_start(
            f1t[64 * g:64 * (g + 1), :, :, :, :],
            f1[:, :, GH * g:GH * (g + 1), :, :].rearrange(
                "b c y x z -> c b y (x z)"))

    # Two phases per batch: dz in {0, 2} shifts read the plain f2 copy, the
    # dz=1 phase reads the z-shifted copy.  One f2 halo tile live per phase.
    for b in range(B):
        for src, s_ranges in ((f2a, [(0, NSX2), (2 * NSX2, 3 * NSX2)]),
                              (f2b, [(NSX2, 2 * NSX2)])):
            f2t = f2_pool.tile([128, F2Y, F2X, F2Z], BF16, tag="f2")
            for g in range(2):
                nc.gpsimd.dma_start(
                    f2t[64 * g:64 * (g + 1), :, :, :],
                    src[b, :, GH * g:GH * g + F2Y, :, :])
            for r0, r1 in s_ranges:
                for s0 in range(r0, r1, SGRP):
                    sg = min(SGRP, r1 - s0)
                    last_q = NCHUNK * sg - 1
                    bank = psum_pool.tile([128, CHUNK], F32)
                    for si in range(sg):
                        s = s0 + si
                        dzi, rem = divmod(s, NSX2)
                        dyi, dxi = divmod(rem, NSX)
                        dz0 = dzi if dzi != 1 else 0
                        ptile = prod_pool.tile([128, GH, W, D], BF16,
                                               tag="prod")
                        nc.vector.tensor_mul(
                            ptile[:, :, :, :], f1t[:, b, :, :, :],
                            f2t[:, dyi:dyi + GH, dxi:dxi + W, dz0:dz0 + D])
                        ptf = ptile.rearrange("p y x z -> p (y x z)")
                        for cch in range(NCHUNK):
                            q = si * NCHUNK + cch
                            jj, t = divmod(q, 16)
                            nc.tensor.matmul(
                                bank[32 * jj:32 * jj + 32, :],
                                wt[:, 32 * t:32 * t + 32],
                                ptf[:, CHUNK * cch:CHUNK * (cch + 1)],
                                start=(t == 0), stop=(t == 15 or q == last_q),
                                tile_position=(0, 32 * jj),
                            )
                    rows = 18 * sg
                    ctile = stage_pool.tile([128, CHUNK], F32)
                    nc.scalar.copy(ctile[:rows, :], bank[:rows, :])
                    for si in range(sg):
                        hbm = cost[b, s0 + si].rearrange(
                            "y x z -> (y x z)").rearrange(
                            "(g cj ci) -> cj g ci", g=2, cj=NCHUNK)
                        nc.sync.dma_start(hbm, ctile[18 * si:18 * si + 18, :])
                    next(mask_iter, None)
    # Drain any leftover mask work.
    for _ in mask_iter:
        pass


def build_program():
    nc = bacc.Bacc("TRN2", target_bir_lowering=False, debug=False,
                   num_devices=NCORES)
    f1 = nc.dram_tensor("f1", [B, C, HS, W, D], BF16, kind="ExternalInput").ap()
    f2a = nc.dram_tensor("f2a", [B, C, HP, F2X, F2Z], BF16,
                         kind="ExternalInput").ap()
    f2b = nc.dram_tensor("f2b", [B, C, HP, F2X, F2Z], BF16,
                         kind="ExternalInput").ap()
    a1 = nc.dram_tensor("a1", [B, HS, W, D], F32, kind="ExternalInput").ap()
    a2py = nc.dram_tensor("a2py", [B, HP, W, D], F32,
                          kind="ExternalInput").ap()
    wones = nc.dram_tensor("wones", [128, CHUNK], BF16,
                           kind="ExternalInput").ap()
    cost = nc.dram_tensor("cost", [B, NS, HS, W, D], F32,
                          kind="ExternalOutput").ap()
    cmask = nc.dram_tensor("cmask", [B, NS, HS, W, D], F32,
                           kind="ExternalOutput").ap()

    with tile.TileContext(nc) as tc:
        with ExitStack() as ctx:
            mask_iter = _mask_steps(nc, tc, ctx, a1, a2py, cmask)
            _build_cost_path(nc, tc, ctx, f1, f2a, f2b, wones, cost,
                             mask_iter)
    nc.compile()
    return nc


def make_wones() -> np.ndarray:
    """Ones-pattern PE weights: column t*32 + r is 1 on channel-group g rows
    iff r == 2t + g, so accumulation step t lands chunk t's two group sums on
    strip-local rows 2t, 2t+1."""
    w = np.zeros((128, CHUNK), np.float32)
    for t in range(16):
        for g in range(2):
            w[g * 64:(g + 1) * 64, t * 32 + 2 * t + g] = 1.0
    return w.astype(ml_dtypes.bfloat16)


def prepare_host_inputs(mpi1_features, mpi1_alpha, mpi2_features, mpi2_alpha):
    f1 = np.asarray(mpi1_features, np.float32)
    a1 = np.asarray(mpi1_alpha, np.float32)[:, 0]          # [B, H, W, D]
    f2 = np.asarray(mpi2_features, np.float32)
    a2 = np.asarray(mpi2_alpha, np.float32)[:, 0]

    bf = ml_dtypes.bfloat16
    # Fold the 1/64 channel mean into f1's bf16 cast (exact, power of two).
    f1b = (f1 * (1.0 / C)).astype(bf)
    f2p = np.zeros((B, C, H + 2 * S, W + 2 * S, D + 2 * SD), np.float32)
    f2p[:, :, S:S + H, S:S + W, SD:SD + D] = f2
    f2pa = f2p.astype(bf)
    f2pb = np.zeros_like(f2p)
    f2pb[..., :-1] = f2p[..., 1:]                           # z-shift by +1
    f2pb = f2pb.astype(bf)
    # alpha2 with y padded by 0.125: padded rows z-sum to exactly 1.0.
    a2py = np.full((B, H + 2 * S, W, D), 0.125, np.float32)
    a2py[:, S:S + H] = a2
    return f1b, f2pa, f2pb, a1, a2py


def make_in_maps(f1b, f2pa, f2pb, a1, a2py):
    wones = make_wones()
    in_maps = []
    for k in range(NCORES):
        y0 = k * HS
        in_maps.append({
            "f1": np.ascontiguousarray(f1b[:, :, y0:y0 + HS]),
            "f2a": np.ascontiguousarray(f2pa[:, :, y0:y0 + HP]),
            "f2b": np.ascontiguousarray(f2pb[:, :, y0:y0 + HP]),
            "a1": np.ascontiguousarray(a1[:, y0:y0 + HS]),
            "a2py": np.ascontiguousarray(a2py[:, y0:y0 + HP]),
            "wones": wones,
        })
    return in_maps


_PROGRAM_CACHE = {}


def kernel(mpi1_features, mpi1_alpha, mpi2_features, mpi2_alpha,
           _trace=False, _trace_kwargs=None):
    if "nc" not in _PROGRAM_CACHE:
        _PROGRAM_CACHE["nc"] = build_program()
    nc = _PROGRAM_CACHE["nc"]
    in_maps = make_in_maps(*prepare_host_inputs(
        mpi1_features, mpi1_alpha, mpi2_features, mpi2_alpha))
    res = run_bass_kernel_spmd(nc, in_maps, list(range(NCORES)),
                               trace=_trace, **(_trace_kwargs or {}))
    cost = np.concatenate([res.results[k]["cost"] for k in range(NCORES)],
                          axis=2)
    cmask = np.concatenate([res.results[k]["cmask"] for k in range(NCORES)],
                           axis=2)
    if _trace:
        kernel.last_results = res
    return cost, cmask


# revision 23
# speedup vs baseline: 1.0035x; 1.0035x over previous
"""Trainium2 Bass kernel for a FlowNet-style MPI correlation layer.

Reference computation (per batch b, shift s=(k,i,j), position p=(y,x,z)):
    cost[b,s,p]  = mean_c f1[b,c,p] * f2pad[b,c,p+delta_s]        (243 shifts)
    cmask[b,s,p] = clip(mask1[b,p] * m2pad[b,p+delta_s], 0, 1)
with mask1 = clip(sum_z alpha1, 0, 1) broadcast over z, f2 zero-padded,
m2 one-padded.

Strategy (8 NeuronCores, spatial shard over h: 12 rows/core):
  - Cost path: bf16 elementwise products on VectorE (2x_1P perf mode), with
    c=64 channels on the partition axis packed as (y-half, c) -> 128
    partitions.  Channel reduction runs on TensorE: ones-pattern weights
    reduce each 64-row channel group, with 16 accumulating matmuls packing a
    full 128x512 PSUM bank (32-row strips via tile_position).  ScalarE copies
    PSUM->SBUF; the 1/64 mean scale is folded into the bf16 cast of f1 on the
    host (exact: power of two).
  - dz=+1 shifts would mis-align bf16 pair-packed reads (odd element offset),
    so the host ships a second f2 copy pre-shifted by one z element; shifts
    are processed in dz-phases so one f2 halo buffer is live at a time and
    refills double-buffer.
  - Mask path: bf16 muls with (b, y) on partitions; dy shifts are realized
    with small SBUF->SBUF partition-shift DMAs, dx/dz as free-dim offsets;
    ScalarE upcasts to fp32 before the store.  Mask work is interleaved
    between cost groups in program order so either path fills the other's
    stalls on VectorE.
"""

import numpy as np
import ml_dtypes
from contextlib import ExitStack

import concourse.bacc as bacc
import concourse.tile as tile
from concourse import mybir
from concourse.bass_utils import run_bass_kernel_spmd

# Problem shape (hardcoded per contest contract).
B, C, H, W, D = 4, 64, 96, 96, 8
S, SD = 4, 1                       # spatial / depth search range
NSX = 2 * S + 1                    # 9 shifts per spatial axis
NSD = 2 * SD + 1                   # 3 depth shifts
NS = NSX * NSX * NSD               # 243 total shifts
NSX2 = NSX * NSX
NCORES = 8
HS = H // NCORES                   # 12 rows of y per core
GH = HS // 2                       # 6 = y-half height (partition packing)
HP = HS + 2 * S                    # 20 = y rows incl halo
F2Y, F2X, F2Z = GH + 2 * S, W + 2 * S, D + 2 * SD   # 14, 104, 10
POS = GH * W * D                   # 4608 free positions per product tile
CHUNK = 512
NCHUNK = POS // CHUNK              # 9 matmul chunks per product tile
SGRP = 7                           # shifts per PSUM bank fill (18*7=126 rows)
MSUB = 3                           # mask shifts per staging tile

F32 = mybir.dt.float32
BF16 = mybir.dt.bfloat16


def _mask_steps(nc, tc, ctx, a1, a2py, cmask):
    """Generator: emits mask-path setup, then yields after each 3-shift
    subgroup (81 yields).  Driven interleaved from the cost loop."""
    singles = ctx.enter_context(tc.tile_pool(name="msk_singles", bufs=1))
    m2y_pool = ctx.enter_context(tc.tile_pool(name="msk_m2y", bufs=2))
    mstg_pool = ctx.enter_context(tc.tile_pool(name="msk_stg", bufs=2))

    # mask1 = clip(sum_z alpha1, 0, 1), broadcast over z. partitions b*12+y.
    a1t = singles.tile([B * HS, W, D], F32)
    for b in range(B):
        nc.sync.dma_start(a1t[b * HS:(b + 1) * HS, :, :], a1[b])
    mask1 = singles.tile([B * HS, W], F32)
    nc.vector.tensor_reduce(mask1[:, :], a1t[:, :, :],
                            axis=mybir.AxisListType.X, op=mybir.AluOpType.add)
    nc.vector.tensor_scalar_min(mask1[:, :], mask1[:, :], 1.0)
    mask1b = singles.tile([B * HS, W, D], BF16)
    for zi in range(D):
        nc.vector.tensor_copy(mask1b[:, :, zi], mask1[:, :])

    # m2t: one-padded mask2 halo, partitions b*20+y' (y' in padded grid).
    # a2py is alpha2 with y pre-padded by 0.125 (padded rows z-sum to 1.0);
    # x/z pads come from the memset(1.0).
    a2t = singles.tile([B * HP, W, D], F32)
    for b in range(B):
        nc.sync.dma_start(a2t[b * HP:(b + 1) * HP, :, :], a2py[b])
    m2sum = singles.tile([B * HP, W], F32)
    nc.vector.tensor_reduce(m2sum[:, :], a2t[:, :, :],
                            axis=mybir.AxisListType.X, op=mybir.AluOpType.add)
    nc.vector.tensor_scalar_min(m2sum[:, :], m2sum[:, :], 1.0)
    m2t = singles.tile([B * HP, F2X, F2Z], F32)
    nc.vector.memset(m2t[:, :, :], 1.0)
    for zi in range(D):
        nc.vector.tensor_copy(m2t[:, S:S + W, SD + zi], m2sum[:, :])
    # bf16 copies: plain + z-shifted-by-1 (keeps pair-packed 2x DVE mode
    # for odd dz reads).
    m2tb = singles.tile([B * HP, F2X, F2Z], BF16)
    m2tbz = singles.tile([B * HP, F2X, F2Z], BF16)
    nc.vector.tensor_copy(m2tb[:, :, :], m2t[:, :, :])
    nc.vector.tensor_copy(m2tbz[:, :, :F2Z - 1], m2t[:, :, 1:])
    nc.vector.memset(m2tbz[:, :, F2Z - 1], 1.0)

    tiles = {}

    def load(dyi):
        m2yt = m2y_pool.tile([B * HS, F2X, F2Z], BF16, tag="m2ya")
        m2ytz = m2y_pool.tile([B * HS, F2X, F2Z], BF16, tag="m2yb")
        for b in range(B):
            nc.gpsimd.dma_start(m2yt[b * HS:(b + 1) * HS, :, :],
                                m2tb[b * HP + dyi:b * HP + dyi + HS, :, :])
            nc.gpsimd.dma_start(m2ytz[b * HS:(b + 1) * HS, :, :],
                                m2tbz[b * HP + dyi:b * HP + dyi + HS, :, :])
        tiles[dyi] = (m2yt, m2ytz)

    load(0)
    for dyi in range(NSX):
        if dyi + 1 < NSX:
            load(dyi + 1)
        m2yt, m2ytz = tiles.pop(dyi)
        for dzi in range(NSD):
            msrc, dz0 = (m2yt, dzi) if dzi != 1 else (m2ytz, 0)
            for dx0 in range(0, NSX, MSUB):
                mstg = mstg_pool.tile([B * HS, MSUB, W, D], BF16, tag="mstgb")
                for dd in range(MSUB):
                    nc.vector.tensor_mul(
                        mstg[:, dd, :, :], mask1b[:, :, :],
                        msrc[:, dx0 + dd:dx0 + dd + W, dz0:dz0 + D])
                mstf = mstg_pool.tile([B * HS, MSUB, W, D], F32, tag="mstgf")
                nc.scalar.copy(mstf[:, :, :, :], mstg[:, :, :, :])
                s0 = dzi * NSX2 + dyi * NSX + dx0
                for b in range(B):
                    hbm = cmask[b, s0:s0 + MSUB].rearrange(
                        "s y x z -> y s (x z)")
                    nc.sync.dma_start(
                        hbm, mstf[b * HS:(b + 1) * HS].rearrange(
                            "p s x z -> p s (x z)"))
                yield


def _build_cost_path(nc, tc, ctx, f1, f2a, f2b, wones, cost, mask_iter):
    """Cost volume: bf16 muls on DVE, channel-reduce on PE.  Drives one mask
    subgroup per cost group so the two paths interleave on VectorE."""
    singles = ctx.enter_context(tc.tile_pool(name="cst_singles", bufs=1))
    f2_pool = ctx.enter_context(tc.tile_pool(name="cst_f2", bufs=2))
    prod_pool = ctx.enter_context(tc.tile_pool(name="cst_prod", bufs=3))
    psum_pool = ctx.enter_context(
        tc.tile_pool(name="cst_psum", bufs=4, space="PSUM"))
    stage_pool = ctx.enter_context(tc.tile_pool(name="cst_stage", bufs=3))

    wt = singles.tile([128, CHUNK], BF16)
    nc.sync.dma_start(wt[:, :], wones[:, :])

    # f1 resident for all b: partition g*64+c (g = y half), free (b, yl, x, z)
    # Loaded per (b, g) so the first batch's muls start before later batches'
    # slices land.
    f1t = singles.tile([128, B, GH, W, D], BF16)
    for b in range(B):
        for g in range(2):
            nc.sync.dma_start(
                f1t[64 * g:64 * (g + 1), b, :, :, :],
                f1[b, :, GH * g:GH * (g + 1), :, :].rearrange(
                    "c y x z -> c y (x z)"))

    # Two phases per batch: dz in {0, 2} shifts read the plain f2 copy, the
    # dz=1 phase reads the z-shifted copy.  One f2 halo tile live per phase.
    for b in range(B):
        for src, s_ranges in ((f2a, [(0, NSX2), (2 * NSX2, 3 * NSX2)]),
                              (f2b, [(NSX2, 2 * NSX2)])):
            f2t = f2_pool.tile([128, F2Y, F2X, F2Z], BF16, tag="f2")
            for g in range(2):
                nc.gpsimd.dma_start(
                    f2t[64 * g:64 * (g + 1), :, :, :],
                    src[b, :, GH * g:GH * g + F2Y, :, :])
            for r0, r1 in s_ranges:
                for s0 in range(r0, r1, SGRP):
                    sg = min(SGRP, r1 - s0)
                    last_q = NCHUNK * sg - 1
                    bank = psum_pool.tile([128, CHUNK], F32)
                    for si in range(sg):
                        s = s0 + si
                        dzi, rem = divmod(s, NSX2)
                        dyi, dxi = divmod(rem, NSX)
                        dz0 = dzi if dzi != 1 else 0
                        ptile = prod_pool.tile([128, GH, W, D], BF16,
                                               tag="prod")
                        nc.vector.tensor_mul(
                            ptile[:, :, :, :], f1t[:, b, :, :, :],
                            f2t[:, dyi:dyi + GH, dxi:dxi + W, dz0:dz0 + D])
                        ptf = ptile.rearrange("p y x z -> p (y x z)")
                        for cch in range(NCHUNK):
                            q = si * NCHUNK + cch
                            jj, t = divmod(q, 16)
                            nc.tensor.matmul(
                                bank[32 * jj:32 * jj + 32, :],
                                wt[:, 32 * t:32 * t + 32],
                                ptf[:, CHUNK * cch:CHUNK * (cch + 1)],
                                start=(t == 0), stop=(t == 15 or q == last_q),
                                tile_position=(0, 32 * jj),
                            )
                    rows = 18 * sg
                    ctile = stage_pool.tile([128, CHUNK], F32)
                    nc.scalar.copy(ctile[:rows, :], bank[:rows, :])
                    for si in range(sg):
                        hbm = cost[b, s0 + si].rearrange(
                            "y x z -> (y x z)").rearrange(
                            "(g cj ci) -> cj g ci", g=2, cj=NCHUNK)
                        nc.sync.dma_start(hbm, ctile[18 * si:18 * si + 18, :])
                    next(mask_iter, None)
    # Drain any leftover mask work.
    for _ in mask_iter:
        pass


def build_program():
    nc = bacc.Bacc("TRN2", target_bir_lowering=False, debug=False,
                   num_devices=NCORES)
    f1 = nc.dram_tensor("f1", [B, C, HS, W, D], BF16, kind="ExternalInput").ap()
    f2a = nc.dram_tensor("f2a", [B, C, HP, F2X, F2Z], BF16,
                         kind="ExternalInput").ap()
    f2b = nc.dram_tensor("f2b", [B, C, HP, F2X, F2Z], BF16,
                         kind="ExternalInput").ap()
    a1 = nc.dram_tensor("a1", [B, HS, W, D], F32, kind="ExternalInput").ap()
    a2py = nc.dram_tensor("a2py", [B, HP, W, D], F32,
                          kind="ExternalInput").ap()
    wones = nc.dram_tensor("wones", [128, CHUNK], BF16,
                           kind="ExternalInput").ap()
    cost = nc.dram_tensor("cost", [B, NS, HS, W, D], F32,
                          kind="ExternalOutput").ap()
    cmask = nc.dram_tensor("cmask", [B, NS, HS, W, D], F32,
                           kind="ExternalOutput").ap()

    with tile.TileContext(nc) as tc:
        with ExitStack() as ctx:
            mask_iter = _mask_steps(nc, tc, ctx, a1, a2py, cmask)
            _build_cost_path(nc, tc, ctx, f1, f2a, f2b, wones, cost,
                             mask_iter)
    nc.compile()
    return nc


def make_wones() -> np.ndarray:
    """Ones-pattern PE weights: column t*32 + r is 1 on channel-group g rows
    iff r == 2t + g, so accumulation step t lands chunk t's two group sums on
    strip-local rows 2t, 2t+1."""
    w = np.zeros((128, CHUNK), np.float32)
    for t in range(16):
        for g in range(2):
            w[g * 64:(g + 1) * 64, t * 32 + 2 * t + g] = 1.0
    return w.astype(ml_dtypes.bfloat16)


def prepare_host_inputs(mpi1_features, mpi1_alpha, mpi2_features, mpi2_alpha):
    f1 = np.asarray(mpi1_features, np.float32)
    a1 = np.asarray(mpi1_alpha, np.float32)[:, 0]          # [B, H, W, D]
    f2 = np.asarray(mpi2_features, np.float32)
    a2 = np.asarray(mpi2_alpha, np.float32)[:, 0]

    bf = ml_dtypes.bfloat16
    # Fold the 1/64 channel mean into f1's bf16 cast (exact, power of two).
    f1b = (f1 * (1.0 / C)).astype(bf)
    f2p = np.zeros((B, C, H + 2 * S, W + 2 * S, D + 2 * SD), np.float32)
    f2p[:, :, S:S + H, S:S + W, SD:SD + D] = f2
    f2pa = f2p.astype(bf)
    f2pb = np.zeros_like(f2p)
    f2pb[..., :-1] = f2p[..., 1:]                           # z-shift by +1
    f2pb = f2pb.astype(bf)
    # alpha2 with y padded by 0.125: padded rows z-sum to exactly 1.0.
    a2py = np.full((B, H + 2 * S, W, D), 0.125, np.float32)
    a2py[:, S:S + H] = a2
    return f1b, f2pa, f2pb, a1, a2py


def make_in_maps(f1b, f2pa, f2pb, a1, a2py):
    wones = make_wones()
    in_maps = []
    for k in range(NCORES):
        y0 = k * HS
        in_maps.append({
            "f1": np.ascontiguousarray(f1b[:, :, y0:y0 + HS]),
            "f2a": np.ascontiguousarray(f2pa[:, :, y0:y0 + HP]),
            "f2b": np.ascontiguousarray(f2pb[:, :, y0:y0 + HP]),
            "a1": np.ascontiguousarray(a1[:, y0:y0 + HS]),
            "a2py": np.ascontiguousarray(a2py[:, y0:y0 + HP]),
            "wones": wones,
        })
    return in_maps


_PROGRAM_CACHE = {}


def kernel(mpi1_features, mpi1_alpha, mpi2_features, mpi2_alpha,
           _trace=False, _trace_kwargs=None):
    if "nc" not in _PROGRAM_CACHE:
        _PROGRAM_CACHE["nc"] = build_program()
    nc = _PROGRAM_CACHE["nc"]
    in_maps = make_in_maps(*prepare_host_inputs(
        mpi1_features, mpi1_alpha, mpi2_features, mpi2_alpha))
    res = run_bass_kernel_spmd(nc, in_maps, list(range(NCORES)),
                               trace=_trace, **(_trace_kwargs or {}))
    cost = np.concatenate([res.results[k]["cost"] for k in range(NCORES)],
                          axis=2)
    cmask = np.concatenate([res.results[k]["cmask"] for k in range(NCORES)],
                           axis=2)
    if _trace:
        kernel.last_results = res
    return cost, cmask
